# revision 1
# baseline (speedup 1.0000x reference)
"""Trainium2 Bass kernel for an Exphormer-style GNN attention layer.

Strategy (8-core SPMD, graph/data parallel):
  - Nodes are sharded contiguously across cores (6250 per core, padded to
    49 windows of 128 node slots).
  - Edges are sharded by destination core and grouped into the 128-node
    window of their destination; each (window) group is padded to a
    uniform-per-window tile count (max over cores) so all cores run the
    same program.
  - The host pre-gathers x[src], x[dst] and permutes edge_attr into
    dst-window order, transposed ([channel, edge]) and cast to bf16 so
    every device-side matmul has its stationary operand in natural layout.
  - Per 128-edge tile on device:
      K|V = xsrcT.T @ [Wk|Wv], Q = xdstT.T @ Wq, Ee = eT.T @ We  (PE)
      t2 = K*Ee*Q; s = per-head sum; score = exp(clip(s/4))       (DVE/ACT)
      one-hot A^T built from dst-local ids via is_equal           (GPSIMD)
      [wV|Z] += A^T.T @ [msg|score]  accumulated in PSUM          (PE)
  - Window close: h = x + wV/(Z+eps). Global batchnorm stats via
    mask-weighted ones-matmuls + a tiny AllReduce; FFN per window with
    PE transposes; second batchnorm; one more AllReduce.
"""

import math

import numpy as np
import ml_dtypes

import concourse.bass as bass
import concourse.bacc as bacc
import concourse.tile as tile
import concourse.mybir as mybir

F32 = mybir.dt.float32
BF16 = mybir.dt.bfloat16
MULT = mybir.AluOpType.mult
ADD = mybir.AluOpType.add
SUB = mybir.AluOpType.subtract
MIN = mybir.AluOpType.min
MAX = mybir.AluOpType.max
ISEQ = mybir.AluOpType.is_equal
EXP = mybir.ActivationFunctionType.Exp
RELU = mybir.ActivationFunctionType.Relu
SQRT = mybir.ActivationFunctionType.Sqrt
AXX = mybir.AxisListType.X

BF = ml_dtypes.bfloat16

D = 128      # model dim
H = 8        # heads
DH = 16      # head dim
W = 128      # nodes per window
TE = 128     # edges per tile
EPS_BN = 1e-5
EPS_Z = 1e-6
CHUNK_T = 16  # edge tiles per DMA chunk
GRP = 4       # edge tiles per DVE batch group
CLIP_ON_GPSIMD = False
AT_ON_GPSIMD = False
KEV_BUFS = 1
Q_BUFS = 1
WIN_BUFS = 2
SCX_ON_ACT = False


def build_program(ncores, nwin, t_w, n_total):
    """Build the SPMD Bass program (identical on every core)."""
    ttot = sum(t_w)
    etot = ttot * TE
    npcp = nwin * W
    nc = bacc.Bacc("TRN2")

    # ---- DRAM I/O ----
    xsrcT_d = nc.dram_tensor("xsrcT", [D, etot], BF16, kind="ExternalInput")
    xdstT_d = nc.dram_tensor("xdstT", [D, etot], BF16, kind="ExternalInput")
    eT_d = nc.dram_tensor("eT", [D, etot], BF16, kind="ExternalInput")
    atT_d = nc.dram_tensor("atT", [TE, ttot * W], mybir.dt.float8e4, kind="ExternalInput")
    xnm_d = nc.dram_tensor("x_nm", [npcp, D], F32, kind="ExternalInput")
    wkv_d = nc.dram_tensor("wkv", [D, 2 * D], BF16, kind="ExternalInput")
    wq_d = nc.dram_tensor("wq", [D, D], BF16, kind="ExternalInput")
    we_d = nc.dram_tensor("we", [D, D], BF16, kind="ExternalInput")
    w1_d = nc.dram_tensor("w1", [D, 2 * D], BF16, kind="ExternalInput")
    # w2 packed as [128, 256]: [:, 0:128] = W2[0:128, :], [:, 128:256] = W2[128:256, :]
    w2_d = nc.dram_tensor("w2", [D, 2 * D], BF16, kind="ExternalInput")
    b1t_d = nc.dram_tensor("b1t", [D, 2], F32, kind="ExternalInput")
    masks_d = nc.dram_tensor("masks", [W, nwin], F32, kind="ExternalInput")
    # bnvec: [g1 | b1 | g2 | b2 | b2ffn] each [1,128]
    bnvec_d = nc.dram_tensor("bnvec", [1, 5 * D], F32, kind="ExternalInput")
    identb_d = nc.dram_tensor("identb", [128, 128], BF16, kind="ExternalInput")
    identf_d = nc.dram_tensor("identf", [128, 128], F32, kind="ExternalInput")
    out_d = nc.dram_tensor("out", [npcp, D], F32, kind="ExternalOutput")

    inv_n = 1.0 / float(n_total)
    CLIP_ENG = nc.gpsimd if CLIP_ON_GPSIMD else nc.vector
    AT_ENG = nc.gpsimd if AT_ON_GPSIMD else nc.vector

    with tile.TileContext(nc) as tc:
        with (
            tc.tile_pool(name="const", bufs=1) as cpool,
            tc.tile_pool(name="resid", bufs=1) as rpool,
            tc.tile_pool(name="dram", bufs=1, space="DRAM") as dpool,
        ):
            # ---- constants to SBUF ----
            wkv = cpool.tile([D, 2 * D], BF16)
            wq = cpool.tile([D, D], BF16)
            we = cpool.tile([D, D], BF16)
            w1 = cpool.tile([D, 2 * D], BF16)
            w2 = cpool.tile([D, 2 * D], BF16)
            b1t = cpool.tile([D, 2], F32)
            masks = cpool.tile([W, nwin], F32)
            bnvec = cpool.tile([1, 5 * D], F32)
            identb = cpool.tile([128, 128], BF16)
            identf = cpool.tile([128, 128], F32)
            for t, d in [(wkv, wkv_d), (wq, wq_d), (we, we_d), (w1, w1_d),
                         (w2, w2_d), (b1t, b1t_d),
                         (masks, masks_d), (bnvec, bnvec_d), (identb, identb_d),
                         (identf, identf_d)]:
                nc.scalar.dma_start(t[:], d[:])
            onesr = cpool.tile([1, 128], F32)
            nc.vector.memset(onesr[:], 1.0)
            zb = cpool.tile([128, 1], F32)
            nc.vector.memset(zb[:], 0.0)
            # prime each engine's vector clock on the const DMAs so steady-state
            # ops don't accumulate several DMA sem waits (ISA wait-count limit)
            prime = cpool.tile([TE, 1], F32)
            nc.gpsimd.tensor_scalar_add(prime[:], masks[:, 0:1], 0.0)
            primef = cpool.tile([TE, 1], F32)
            nc.vector.tensor_scalar_add(primef[:], identf[:, 0:1], 0.0)
            nc.vector.tensor_scalar_add(primef[:], masks[:, 0:1], 0.0)
            nc.scalar.activation(primef[:], b1t[:, 0:1],
                                 mybir.ActivationFunctionType.Copy)
            nc.scalar.activation(primef[0:1, :], bnvec[:, 0:1],
                                 mybir.ActivationFunctionType.Copy)
            with tc.tile_pool(name="pprime", bufs=1, space="PSUM") as pprime:
                pscr = pprime.tile([1, 1], F32)
                for a, b in [(wkv, wq), (we, w1), (w2, identb)]:
                    nc.tensor.matmul(pscr[:], a[:, 0:1], b[:, 0:1])

            h_all = rpool.tile([W, nwin * D], F32)    # node-major h (resident)
            h3_all = rpool.tile([W, nwin * D], F32)   # node-major h3 (resident)

            # load ALL of local x up-front (residual), as window tiles
            x_all = rpool.tile([W, nwin * D], F32)
            nc.scalar.dma_start(
                x_all[:].rearrange("p (w c) -> p w c", w=nwin),
                xnm_d[:].rearrange("(w p) c -> p w c", p=W))

            # ================= Phase 1: edge phase =================
            with (
                tc.tile_pool(name="chunks", bufs=3) as chpool,
                tc.tile_pool(name="esmall", bufs=4) as espool,
                tc.tile_pool(name="pk", bufs=2, space="PSUM") as pk,
                tc.tile_pool(name="pq", bufs=2, space="PSUM") as pq,
                tc.tile_pool(name="pv", bufs=1, space="PSUM") as pv,
                tc.tile_pool(name="pe_", bufs=1, space="PSUM") as pe_,
                tc.tile_pool(name="pwin", bufs=2, space="PSUM") as pwin,
            ):
                nchunks = (ttot + CHUNK_T - 1) // CHUNK_T
                chunk_tiles = [None] * nchunks

                def get_chunk(gi):
                    ci, j = divmod(gi, CHUNK_T)
                    if chunk_tiles[ci] is None:
                        csz = min(CHUNK_T, ttot - ci * CHUNK_T) * TE
                        xs = chpool.tile([D, csz], BF16, tag="xs")
                        xd = chpool.tile([D, csz], BF16, tag="xd")
                        et = chpool.tile([D, csz], BF16, tag="et")
                        ntl = csz // TE
                        att = chpool.tile([TE, ntl * W], mybir.dt.float8e4, tag="att")
                        c0 = ci * CHUNK_T * TE
                        nc.sync.dma_start(xs[:], xsrcT_d[:, c0:c0 + csz])
                        nc.sync.dma_start(xd[:], xdstT_d[:, c0:c0 + csz])
                        nc.sync.dma_start(et[:], eT_d[:, c0:c0 + csz])
                        nc.sync.dma_start(
                            att[:], atT_d[:, ci * CHUNK_T * W:ci * CHUNK_T * W + ntl * W])
                        chunk_tiles[ci] = (xs, xd, et, att)
                    return chunk_tiles[ci], j

                g = 0
                for w in range(nwin):
                    win = pwin.tile([W, D + H], F32, tag="win")
                    tw = t_w[w]
                    t = 0
                    while t < tw:
                        grp = min(GRP, tw - t)
                        k4 = pk.tile([TE, grp * D], F32, tag="k4")
                        q4 = pq.tile([TE, grp * D], F32, tag="q4")
                        v4 = pv.tile([TE, grp * D], F32, tag="v4")
                        e4 = pe_.tile([TE, grp * D], F32, tag="e4")
                        for j in range(grp):
                            (xs, xd, et, att), cj = get_chunk(g + j)
                            sl = slice(cj * TE, (cj + 1) * TE)
                            js = slice(j * D, (j + 1) * D)
                            nc.tensor.matmul(k4[:, js], xs[:, sl], wkv[:, 0:D])
                            nc.tensor.matmul(v4[:, js], xs[:, sl], wkv[:, D:2 * D])
                            nc.tensor.matmul(q4[:, js], xd[:, sl], wq[:])
                            nc.tensor.matmul(e4[:, js], et[:, sl], we[:])
                        # batched edge elementwise; E/Q evicted on ACT (walrus
                        # allows at most one PSUM operand per DVE op)
                        esb = espool.tile([TE, grp * D], BF16, tag="esb")
                        nc.scalar.activation(
                            esb[:], e4[:], mybir.ActivationFunctionType.Copy)
                        qsb = espool.tile([TE, grp * D], BF16, tag="qsb")
                        nc.scalar.activation(
                            qsb[:], q4[:], mybir.ActivationFunctionType.Copy)
                        t1 = espool.tile([TE, grp * D], BF16, tag="t1")
                        nc.vector.tensor_tensor(t1[:], k4[:], esb[:], op=MULT)
                        t2 = espool.tile([TE, grp * D], BF16, tag="t2")
                        nc.vector.tensor_tensor(t2[:], t1[:], qsb[:], op=MULT)
                        s4 = espool.tile([TE, grp * H], BF16, tag="s4")
                        with nc.allow_low_precision("score logit rounding ok"):
                            nc.vector.tensor_reduce(
                                s4[:], t2[:].rearrange("p (g h d) -> p g h d",
                                                       g=grp, h=H),
                                axis=AXX, op=ADD)
                        CLIP_ENG.tensor_scalar(s4[:], s4[:], 20.0, -20.0,
                                               op0=MIN, op1=MAX)
                        ms4 = espool.tile([TE, grp * (D + H)], BF16, tag="ms4")
                        ms4g = ms4[:].rearrange("p (g x) -> p g x", g=grp)
                        # score, twice: packed [TE,grp*8] slab for Z-aggregation
                        # + expanded [TE,grp*128] copy so msg runs in DVE 2x mode
                        nc.scalar.activation(
                            ms4g[:, :, D:D + H],
                            s4[:].rearrange("p (g h) -> p g h", g=grp),
                            EXP, bias=zb[:], scale=0.25)
                        # expanded score (gpsimd bcast copy) so the msg
                        # multiply runs in DVE 2x mode; V evicted on ACT
                        scx = espool.tile([TE, grp * D], BF16, tag="scx")
                        nc.gpsimd.tensor_copy(
                            scx[:].rearrange("p (g h d) -> p g h d", g=grp, h=H),
                            ms4g[:, :, D:D + H].unsqueeze(3)
                            .broadcast_to([TE, grp, H, DH]))
                        vsb = espool.tile([TE, grp * D], BF16, tag="vsb")
                        nc.scalar.activation(
                            vsb[:], v4[:], mybir.ActivationFunctionType.Copy)
                        nc.vector.tensor_tensor(
                            ms4g[:, :, 0:D],
                            vsb[:].rearrange("p (g c) -> p g c", g=grp),
                            scx[:].rearrange("p (g c) -> p g c", g=grp),
                            op=MULT)
                        for j in range(grp):
                            (xs, xd, et, att), cj = get_chunk(g + j)
                            nc.tensor.matmul(
                                win[:], att[:, cj * W:(cj + 1) * W],
                                ms4[:, j * (D + H):(j + 1) * (D + H)],
                                start=(t + j == 0), stop=(t + j == tw - 1))
                        gl = g + grp - 1
                        if gl % CHUNK_T == CHUNK_T - 1 or gl == ttot - 1:
                            chunk_tiles[gl // CHUNK_T] = None
                        g += grp
                        t += grp

                    # ---- window close: h = x + wV/(Z+eps); h^2 for bn1 stats
                    zi = espool.tile([W, H], F32, tag="zi")
                    nc.scalar.activation(zi[:], win[:, D:D + H],
                                         mybir.ActivationFunctionType.Copy,
                                         bias=EPS_Z)
                    nc.vector.reciprocal(zi[:], zi[:])
                    hw = h_all[:, w * D:(w + 1) * D]
                    nc.vector.tensor_tensor(
                        hw.rearrange("p (h d) -> p h d", h=H),
                        win[:, 0:D].rearrange("p (h d) -> p h d", h=H),
                        zi[:].unsqueeze(2).broadcast_to([W, H, DH]), op=MULT)
                    nc.gpsimd.tensor_tensor(hw, hw,
                                            x_all[:, w * D:(w + 1) * D], op=ADD)
                    nc.gpsimd.tensor_tensor(h3_all[:, w * D:(w + 1) * D],
                                            hw, hw, op=MULT)

            # ================= node phase =================
            with tc.tile_pool(name="nsmall", bufs=3) as nspool:

                def bn_coefs(gstat_ap, g_ap, b_ap, tag):
                    mu = nspool.tile([1, D], F32, tag=tag + "mu")
                    nc.vector.tensor_scalar_mul(mu[:], gstat_ap[:, 0:D], inv_n)
                    var = nspool.tile([1, D], F32, tag=tag + "var")
                    nc.vector.tensor_scalar_mul(var[:], gstat_ap[:, D:2 * D], inv_n)
                    mu2 = nspool.tile([1, D], F32, tag=tag + "mu2")
                    nc.vector.tensor_tensor(mu2[:], mu[:], mu[:], op=MULT)
                    nc.vector.tensor_tensor(var[:], var[:], mu2[:], op=SUB)
                    nc.vector.tensor_scalar_add(var[:], var[:], EPS_BN)
                    sd = nspool.tile([1, D], F32, tag=tag + "sd")
                    nc.scalar.activation(sd[:], var[:], SQRT, bias=zb[0:1, :])
                    nc.vector.reciprocal(sd[:], sd[:])
                    sg = nspool.tile([1, D], F32, tag=tag + "sg")
                    nc.vector.tensor_tensor(sg[:], sd[:], g_ap, op=MULT)
                    bb = nspool.tile([1, D], F32, tag=tag + "bb")
                    nc.vector.tensor_tensor(bb[:], mu[:], sg[:], op=MULT)
                    nc.vector.tensor_tensor(bb[:], b_ap, bb[:], op=SUB)
                    return sg, bb

                def stats_mms(src_all, sq_all, pstat, tag):
                    """masked per-window sums of h and (precomputed) h^2."""
                    ph = pstat.tile([1, D], F32, tag=tag + "ph")
                    ph2 = pstat.tile([1, D], F32, tag=tag + "ph2")
                    for w in range(nwin):
                        hw = src_all[:, w * D:(w + 1) * D]
                        sq = sq_all[:, w * D:(w + 1) * D]
                        st = (w == 0)
                        sp = (w == nwin - 1)
                        nc.tensor.matmul(ph[:], masks[:, w:w + 1], hw,
                                         start=st, stop=sp)
                        nc.tensor.matmul(ph2[:], masks[:, w:w + 1], sq,
                                         start=st, stop=sp)
                    return ph, ph2

                def stats_reduce(ph, ph2, pbc, coef_g, coef_b, extra, tag):
                    """AllReduce stats -> bn coefs, broadcast
                    [scale | bias | *extra] to 128 partitions."""
                    stats = nspool.tile([1, 2 * D], F32, tag=tag + "stats")
                    nc.vector.tensor_copy(stats[:, 0:D], ph[:])
                    nc.vector.tensor_copy(stats[:, D:2 * D], ph2[:])
                    cc_in = dpool.tile([1, 2 * D], F32, tag=tag + "ccin")
                    cc_out = dpool.tile([1, 2 * D], F32, tag=tag + "ccout")
                    nc.scalar.dma_start(cc_in[:], stats[:])
                    nc.gpsimd.collective_compute(
                        "AllReduce", ADD, replica_groups=[list(range(ncores))],
                        ins=[cc_in.opt()], outs=[cc_out.opt()])
                    gstat = nspool.tile([1, 2 * D], F32, tag=tag + "gstat")
                    nc.scalar.dma_start(gstat[:], cc_out[:])
                    sg, bb = bn_coefs(gstat, coef_g, coef_b, tag)
                    nex = 2 + len(extra)
                    brc = nspool.tile([1, nex * D], F32, tag=tag + "brc")
                    nc.vector.tensor_copy(brc[:, 0:D], sg[:])
                    nc.vector.tensor_copy(brc[:, D:2 * D], bb[:])
                    for i, ex in enumerate(extra):
                        nc.vector.tensor_copy(brc[:, (2 + i) * D:(3 + i) * D], ex)
                    pb = pbc.tile([128, nex * D], F32, tag=tag + "pb")
                    nc.tensor.matmul(pb[:], onesr[:], brc[:])
                    bc = nspool.tile([128, nex * D], F32, tag=tag + "bc")
                    nc.vector.tensor_copy(bc[:], pb[:])
                    return bc, sg, bb

                def bcast_win(vec_ap):
                    """[1(or 128), D] -> broadcast over windows vs [W, nwin*D]."""
                    return vec_ap.unsqueeze(1).broadcast_to([128, nwin, D])

                h_allw = h_all[:].rearrange("p (w c) -> p w c", w=nwin)
                h3_allw = h3_all[:].rearrange("p (w c) -> p w c", w=nwin)
                # h transposed (bf16) for the FFN, done BEFORE/OVERLAPPING the
                # bn1 AllReduce: bn1's affine folds into W1 rows (scale by s1)
                # and the relu bias; the per-channel shifts (b1', b2) are
                # invariant under bn2 and are dropped from the residual path.
                hT_all = rpool.tile([D, nwin * W], BF16)

                with (
                    tc.tile_pool(name="pstat1", bufs=1, space="PSUM") as pstat1,
                    tc.tile_pool(name="pbc1", bufs=1, space="PSUM") as pbc1,
                    tc.tile_pool(name="ptr", bufs=2, space="PSUM") as ptr,
                ):
                    # stats first so the AllReduce launches ASAP; the
                    # transposes below then overlap the collective latency
                    ph, ph2 = stats_mms(h_all, h3_all, pstat1, "s1")
                    for w in range(nwin):
                        hw = h_all[:, w * D:(w + 1) * D]
                        hnb = nspool.tile([W, D], BF16, tag="hnb")
                        nc.vector.tensor_copy(hnb[:], hw)
                        pT = ptr.tile([D, W], BF16, tag="pT")
                        nc.tensor.transpose(pT[:], hnb[:], identb[:])
                        nc.vector.tensor_copy(hT_all[:, w * W:(w + 1) * W], pT[:])
                    bc1, sg1, bb1 = stats_reduce(ph, ph2, pbc1,
                                                 bnvec[:, 0:D], bnvec[:, D:2 * D],
                                                 [], "s1")
                    # column versions of s1/b1' via K=1 matmuls
                    s1c_p = pbc1.tile([D, 1], F32, tag="s1cp")
                    nc.tensor.matmul(s1c_p[:], sg1[:], onesr[0:1, 0:1])
                    s1col = nspool.tile([D, 1], F32, tag="s1col")
                    nc.vector.tensor_copy(s1col[:], s1c_p[:])
                    b1c_p = pbc1.tile([D, 1], F32, tag="b1cp")
                    nc.tensor.matmul(b1c_p[:], bb1[:], onesr[0:1, 0:1])
                    b1col = nspool.tile([D, 1], BF16, tag="b1col")
                    nc.vector.tensor_copy(b1col[:], b1c_p[:])
                    # W1' = diag(s1) @ W1  (row scaling on ACT)
                    w1p = nspool.tile([D, 2 * D], BF16, tag="w1p")
                    nc.scalar.activation(w1p[:], w1[:],
                                         mybir.ActivationFunctionType.Copy,
                                         scale=s1col[:])
                    # relu bias: b1' @ W1 + b1, as [D, 2] columns
                    bias2 = nspool.tile([D, 2], F32, tag="bias2")
                    for jj in range(2):
                        bv_p = pbc1.tile([D, 1], F32, tag="bvp")
                        nc.tensor.matmul(bv_p[:], w1[:, jj * D:(jj + 1) * D],
                                         b1col[:])
                        nc.vector.tensor_tensor(bias2[:, jj:jj + 1], bv_p[:],
                                                b1t[:, jj:jj + 1], op=ADD)

                # ---- Phase 4: FFN + residual (h*s1 + h2) + bn2 stats ----
                with tc.tile_pool(name="pstat2", bufs=1, space="PSUM") as pstat2:
                  with tc.tile_pool(name="pffn", bufs=2, space="PSUM") as pffn:
                    p2h = pstat2.tile([1, D], F32, tag="s2ph")
                    p2h2 = pstat2.tile([1, D], F32, tag="s2ph2")
                    for w in range(nwin):
                        hw = h_all[:, w * D:(w + 1) * D]
                        h3w = h3_all[:, w * D:(w + 1) * D]
                        sq2w = x_all[:, w * D:(w + 1) * D]
                        hnT = hT_all[:, w * W:(w + 1) * W]
                        f1a = pffn.tile([D, W], F32, tag="f1a")
                        f1b = pffn.tile([D, W], F32, tag="f1b")
                        nc.tensor.matmul(f1a[:], w1p[:, 0:D], hnT)
                        nc.tensor.matmul(f1b[:], w1p[:, D:2 * D], hnT)
                        ra = nspool.tile([D, W], BF16, tag="ra")
                        rb = nspool.tile([D, W], BF16, tag="rb")
                        nc.scalar.activation(ra[:], f1a[:], RELU, bias=bias2[:, 0:1])
                        nc.scalar.activation(rb[:], f1b[:], RELU, bias=bias2[:, 1:2])
                        # h2 node-major directly: lhsT = r (stationary),
                        # rhs = W2 rows -> out [n, c]; no transpose needed
                        p2 = pffn.tile([W, D], F32, tag="p2")
                        nc.tensor.matmul(p2[:], ra[:], w2[:, 0:D],
                                         start=True, stop=False)
                        nc.tensor.matmul(p2[:], rb[:], w2[:, D:2 * D],
                                         start=False, stop=True)
                        # h3' = h*s1 + h2   (b1', b2 shifts dropped: bn2-invariant)
                        nc.vector.tensor_tensor(h3w, hw, bc1[:, 0:D], op=MULT)
                        nc.vector.tensor_tensor(h3w, h3w, p2[:], op=ADD)
                        nc.gpsimd.tensor_tensor(sq2w, h3w, h3w, op=MULT)
                        st = (w == 0)
                        sp = (w == nwin - 1)
                        nc.tensor.matmul(p2h[:], masks[:, w:w + 1], h3w,
                                         start=st, stop=sp)
                        nc.tensor.matmul(p2h2[:], masks[:, w:w + 1], sq2w,
                                         start=st, stop=sp)
                  with tc.tile_pool(name="pbc2", bufs=1, space="PSUM") as pbc2:
                    bc2, _, _ = stats_reduce(p2h, p2h2, pbc2,
                                             bnvec[:, 2 * D:3 * D],
                                             bnvec[:, 3 * D:4 * D], [], "s2")

                # ---- Phase 6: bn2 apply (batched, into h_all) + one DMA out ----
                PCH = 13
                for w0 in range(0, nwin, PCH):
                    w1_ = min(w0 + PCH, nwin)
                    hs = h_all[:, w0 * D:w1_ * D].rearrange(
                        "p (w c) -> p w c", w=w1_ - w0)
                    h3s = h3_all[:, w0 * D:w1_ * D].rearrange(
                        "p (w c) -> p w c", w=w1_ - w0)
                    nb = w1_ - w0
                    nc.vector.tensor_tensor(
                        hs, h3s, bc2[:, 0:D].unsqueeze(1).broadcast_to([128, nb, D]),
                        op=MULT)
                    nc.vector.tensor_tensor(
                        hs, hs, bc2[:, D:2 * D].unsqueeze(1).broadcast_to([128, nb, D]),
                        op=ADD)
                    nc.scalar.dma_start(
                        out_d[w0 * W:w1_ * W, :].rearrange("(w p) c -> p w c", p=W),
                        hs)

    nc.compile()
    return nc


def host_prepare(x, edge_attr, edge_index, Wq, Wk, We, Wv, bn1_g, bn1_b,
                 W1, b1, W2, b2, bn2_g, bn2_b, ncores):
    """Shard + permute inputs on the host; returns (in_maps, nwin, t_w, N)."""
    N = x.shape[0]
    E = edge_index.shape[1]
    assert N % ncores == 0, (N, ncores)
    npc = N // ncores
    nwin = (npc + W - 1) // W
    npcp = nwin * W

    src = np.asarray(edge_index[0], dtype=np.int64)
    dst = np.asarray(edge_index[1], dtype=np.int64)
    core = dst // npc
    rem = dst - core * npc
    wloc = rem // W
    dloc = rem - wloc * W

    gw = core * nwin + wloc  # global window id in [0, ncores*nwin)
    order = np.argsort(gw, kind="stable")
    gw_s = gw[order]
    counts = np.bincount(gw_s, minlength=ncores * nwin).reshape(ncores, nwin)
    t_w = np.maximum(1, (counts.max(axis=0) + TE - 1) // TE).astype(np.int64)
    ttot = int(t_w.sum())
    etot = ttot * TE

    tile_off = np.concatenate([[0], np.cumsum(t_w)])  # per window, in tiles

    # slot index for every edge: position within its (core, window) group
    starts = np.concatenate([[0], np.cumsum(counts.reshape(-1))])
    within = np.arange(E, dtype=np.int64) - starts[gw_s]
    slot = (core[order] * etot + tile_off[wloc[order]] * TE + within)

    x32 = np.asarray(x, dtype=np.float32)
    ea32 = np.asarray(edge_attr, dtype=np.float32)

    # padded per-core edge-slot arrays
    xsrc = np.zeros((ncores, etot, D), dtype=np.float32)
    xdst = np.zeros((ncores, etot, D), dtype=np.float32)
    eat = np.zeros((ncores, etot, D), dtype=np.float32)
    dlocs = np.full((ncores, etot), -1.0, dtype=np.float32)
    sc_idx = slot // etot
    sl_idx = slot % etot
    xsrc[sc_idx, sl_idx] = x32[src[order]]
    xdst[sc_idx, sl_idx] = x32[dst[order]]
    eat[sc_idx, sl_idx] = ea32[order]
    dlocs[sc_idx, sl_idx] = dloc[order].astype(np.float32)

    identb = np.eye(128, dtype=np.float32).astype(BF)
    identf = np.eye(128, dtype=np.float32)
    wkv = np.concatenate([Wk, Wv], axis=1).astype(BF)
    bnvec = np.concatenate([bn1_g, bn1_b, bn2_g, bn2_b, b2]).reshape(1, 5 * D)
    bnvec = np.ascontiguousarray(bnvec, dtype=np.float32)
    b1t = np.ascontiguousarray(np.asarray(b1, np.float32).reshape(2, D).T)

    masks = np.zeros((W, nwin), dtype=np.float32)
    flat = np.arange(npcp).reshape(nwin, W).T  # [W, nwin] node slot ids
    masks[flat < npc] = 1.0

    in_maps = []
    for c in range(ncores):
        xl = np.zeros((npcp, D), dtype=np.float32)
        xl[:npc] = x32[c * npc:(c + 1) * npc]
        in_maps.append({
            "xsrcT": np.ascontiguousarray(xsrc[c].T).astype(BF),
            "xdstT": np.ascontiguousarray(xdst[c].T).astype(BF),
            "eT": np.ascontiguousarray(eat[c].T).astype(BF),
            "atT": np.ascontiguousarray(
                (dlocs[c].reshape(ttot, TE)[:, :, None]
                 == np.arange(W, dtype=np.float32)[None, None, :])
                .transpose(1, 0, 2).reshape(TE, ttot * W)).astype(ml_dtypes.float8_e4m3),
            "x_nm": xl,
            "wkv": wkv,
            "wq": np.asarray(Wq, np.float32).astype(BF),
            "we": np.asarray(We, np.float32).astype(BF),
            "w1": np.asarray(W1, np.float32).astype(BF),
            "w2": np.concatenate(
                [np.asarray(W2, np.float32)[0:D], np.asarray(W2, np.float32)[D:2 * D]],
                axis=1).astype(BF),
            "b1t": b1t,
            "masks": masks,
            "bnvec": bnvec,
            "identb": identb,
            "identf": identf,
        })
    return in_maps, nwin, [int(t) for t in t_w], N


_CACHE = {}


def _get_program(ncores, nwin, t_w, n_total):
    key = (ncores, nwin, tuple(t_w), n_total)
    if key not in _CACHE:
        _CACHE[key] = build_program(ncores, nwin, t_w, n_total)
    return _CACHE[key]


def kernel(x, edge_attr, edge_index, Wq, Wk, We, Wv, bn1_g, bn1_b,
           W1, b1, W2, b2, bn2_g, bn2_b, _ncores=8, _return_extra=False):
    from concourse.bass_utils import run_bass_kernel_spmd

    in_maps, nwin, t_w, N = host_prepare(
        x, edge_attr, edge_index, Wq, Wk, We, Wv, bn1_g, bn1_b,
        W1, b1, W2, b2, bn2_g, bn2_b, _ncores)
    nc = _get_program(_ncores, nwin, t_w, N)
    res = run_bass_kernel_spmd(nc, in_maps, list(range(_ncores)))
    npc = N // _ncores
    out = np.concatenate([r["out"][:npc] for r in res.results], axis=0)
    out = np.ascontiguousarray(out, dtype=np.float32)
    if _return_extra:
        return out, res
    return out



# revision 14
# speedup vs baseline: 1.0981x; 1.0981x over previous
"""Trainium2 Bass kernel for an Exphormer-style GNN attention layer.

Strategy (8-core SPMD, graph/data parallel):
  - Nodes are sharded contiguously across cores (6250 per core); within a
    core, nodes are greedily packed into 49 windows of <=128 slots so that
    per-window edge counts are balanced (minimizes tile padding).
  - Edges are sharded by destination core and grouped into the window of
    their destination; each window group is padded to a uniform-per-window
    tile count (max over cores) so all cores run the same program.
  - The host pre-gathers x[src], x[dst] and permutes edge_attr into
    dst-window order, transposed ([channel, edge]) and cast to bf16 so
    every device-side matmul has its stationary operand in natural layout.
  - Per 128-edge tile on device (engine-balanced against the TRN2 cost
    model: DVE ~0.52ns/col at 2x, ACT 0.83ns/col + 185ns init, Pool
    0.83/0.42 ns/col + 95ns launch, PSUM operands force 1x + init):
      K,V,Q,E projections on PE into PSUM
      t1 = K*E          on Pool (two PSUM operands, SBUF bf16 out)
      qsb = copy(Q)     on ACT (PSUM -> SBUF bf16)
      t2 = t1*qsb       on DVE (all-SBUF bf16, 2x mode)
      per-head sum      pairwise tree: r1 DVE, r2 Pool, r3+r4 DVE
      clip              on DVE; exp on ACT into the score slab
      msg = V*score     on DVE (V from PSUM, score broadcast fused)
      [wV|Z] += A^T.T @ [msg|score]  accumulated in PSUM (PE)
  - Window close: h = x + wV/(Z+eps) (bf16), h^2 on ACT, bn1 stats
    matmuls ride along per window. Global batchnorm via AllReduce; FFN
    per window with PE transposes; second batchnorm; one DMA out.
"""

import numpy as np
import ml_dtypes

import concourse.bass as bass
import concourse.bacc as bacc
import concourse.tile as tile
import concourse.mybir as mybir

F32 = mybir.dt.float32
BF16 = mybir.dt.bfloat16
MULT = mybir.AluOpType.mult
ADD = mybir.AluOpType.add
SUB = mybir.AluOpType.subtract
MIN = mybir.AluOpType.min
MAX = mybir.AluOpType.max
EXP = mybir.ActivationFunctionType.Exp
RELU = mybir.ActivationFunctionType.Relu
SQRT = mybir.ActivationFunctionType.Sqrt
SQUARE = mybir.ActivationFunctionType.Square
COPY = mybir.ActivationFunctionType.Copy
AXX = mybir.AxisListType.X

BF = ml_dtypes.bfloat16

D = 128      # model dim
H = 8        # heads
DH = 16      # head dim
W = 128      # nodes per window
TE = 128     # edges per tile
EPS_BN = 1e-5
EPS_Z = 1e-6
CHUNK_T = 16  # edge tiles per DMA chunk
GRP = 4       # edge tiles per DVE batch group


def build_program(ncores, nwin, t_w, n_total):
    """Build the SPMD Bass program (identical on every core)."""
    ttot = sum(t_w)
    etot = ttot * TE
    npcp = nwin * W
    nc = bacc.Bacc("TRN2")

    # ---- DRAM I/O ----
    xsrcT_d = nc.dram_tensor("xsrcT", [D, etot], BF16, kind="ExternalInput")
    xdstT_d = nc.dram_tensor("xdstT", [D, etot], BF16, kind="ExternalInput")
    eT_d = nc.dram_tensor("eT", [D, etot], BF16, kind="ExternalInput")
    atT_d = nc.dram_tensor("atT", [TE, ttot * W], mybir.dt.float8e4, kind="ExternalInput")
    xnm_d = nc.dram_tensor("x_nm", [npcp, D], BF16, kind="ExternalInput")
    wk_d = nc.dram_tensor("wk", [D, D], BF16, kind="ExternalInput")
    wv_d = nc.dram_tensor("wv", [D, D], BF16, kind="ExternalInput")
    wq_d = nc.dram_tensor("wq", [D, D], BF16, kind="ExternalInput")
    we_d = nc.dram_tensor("we", [D, D], BF16, kind="ExternalInput")
    w1_d = nc.dram_tensor("w1", [D, 2 * D], BF16, kind="ExternalInput")
    # w2 packed as [128, 256]: [:, 0:128] = W2[0:128, :], [:, 128:256] = W2[128:256, :]
    w2_d = nc.dram_tensor("w2", [D, 2 * D], BF16, kind="ExternalInput")
    b1t_d = nc.dram_tensor("b1t", [D, 2], F32, kind="ExternalInput")
    masks_d = nc.dram_tensor("masks", [W, nwin], BF16, kind="ExternalInput")
    # bnvec: [g1 | b1 | g2 | b2 | b2ffn] each [1,128]
    bnvec_d = nc.dram_tensor("bnvec", [1, 5 * D], F32, kind="ExternalInput")
    identb_d = nc.dram_tensor("identb", [128, 128], BF16, kind="ExternalInput")
    out_d = nc.dram_tensor("out", [npcp, D], BF16, kind="ExternalOutput")

    inv_n = 1.0 / float(n_total)

    with tile.TileContext(nc) as tc:
        with (
            tc.tile_pool(name="const", bufs=1) as cpool,
            tc.tile_pool(name="resid", bufs=1) as rpool,
            tc.tile_pool(name="dram", bufs=1, space="DRAM") as dpool,
        ):
            # ---- constants to SBUF ----
            wk = cpool.tile([D, D], BF16)
            wv = cpool.tile([D, D], BF16)
            wq = cpool.tile([D, D], BF16)
            we = cpool.tile([D, D], BF16)
            w1 = cpool.tile([D, 2 * D], BF16)
            w2 = cpool.tile([D, 2 * D], BF16)
            b1t = cpool.tile([D, 2], F32)
            masks = cpool.tile([W, nwin], BF16)
            bnvec = cpool.tile([1, 5 * D], F32)
            identb = cpool.tile([128, 128], BF16)
            for t, d in [(wk, wk_d), (wv, wv_d), (wq, wq_d), (we, we_d),
                         (w1, w1_d), (w2, w2_d), (b1t, b1t_d),
                         (masks, masks_d), (bnvec, bnvec_d), (identb, identb_d)]:
                nc.scalar.dma_start(t[:], d[:])
            onesr = cpool.tile([1, 128], F32)
            nc.vector.memset(onesr[:], 1.0)
            zb = cpool.tile([128, 1], F32)
            nc.vector.memset(zb[:], 0.0)
            # prime each engine's vector clock on the const DMAs so steady-state
            # ops don't accumulate several DMA sem waits (ISA wait-count limit)
            prime = cpool.tile([TE, 1], F32)
            nc.gpsimd.tensor_scalar_add(prime[:], masks[:, 0:1], 0.0)
            primef = cpool.tile([TE, 1], F32)
            nc.vector.tensor_scalar_add(primef[:], identb[:, 0:1], 0.0)
            nc.vector.tensor_scalar_add(primef[:], masks[:, 0:1], 0.0)
            nc.scalar.activation(primef[:], b1t[:, 0:1], COPY)
            nc.scalar.activation(primef[0:1, :], bnvec[:, 0:1], COPY)
            with tc.tile_pool(name="pprime", bufs=1, space="PSUM") as pprime:
                pscr = pprime.tile([1, 1], F32)
                for a, b in [(wk, wq), (we, w1), (w2, identb)]:
                    nc.tensor.matmul(pscr[:], a[:, 0:1], b[:, 0:1])

            h_all = rpool.tile([W, nwin * D], BF16)    # node-major h (resident)
            h3_all = rpool.tile([W, nwin * D], BF16)   # node-major h3 (resident)

            # load ALL of local x up-front (residual), as window tiles
            x_all = rpool.tile([W, nwin * D], BF16)
            nc.scalar.dma_start(
                x_all[:].rearrange("p (w c) -> p w c", w=nwin),
                xnm_d[:].rearrange("(w p) c -> p w c", p=W))

            # ================= Phase 1: edge phase =================
            with (
                tc.tile_pool(name="chunks", bufs=3) as chpool,
                tc.tile_pool(name="esmall", bufs=4) as espool,
                tc.tile_pool(name="pk", bufs=1, space="PSUM") as pk,
                tc.tile_pool(name="peq", bufs=1, space="PSUM") as peq,
                tc.tile_pool(name="pv", bufs=2, space="PSUM") as pv,
                tc.tile_pool(name="pwin", bufs=2, space="PSUM") as pwin,
                tc.tile_pool(name="pstat1", bufs=1, space="PSUM") as pstat1,
            ):
                phs = pstat1.tile([1, 2 * D], F32, tag="s1ph")
                ph1 = phs[:, 0:D]
                ph2 = phs[:, D:2 * D]
                nchunks = (ttot + CHUNK_T - 1) // CHUNK_T
                chunk_tiles = [None] * nchunks

                def get_chunk(gi):
                    ci, j = divmod(gi, CHUNK_T)
                    if chunk_tiles[ci] is None:
                        csz = min(CHUNK_T, ttot - ci * CHUNK_T) * TE
                        xs = chpool.tile([D, csz], BF16, tag="xs")
                        xd = chpool.tile([D, csz], BF16, tag="xd")
                        et = chpool.tile([D, csz], BF16, tag="et")
                        ntl = csz // TE
                        att = chpool.tile([TE, ntl * W], mybir.dt.float8e4, tag="att")
                        c0 = ci * CHUNK_T * TE
                        nc.sync.dma_start(xs[:], xsrcT_d[:, c0:c0 + csz])
                        nc.sync.dma_start(xd[:], xdstT_d[:, c0:c0 + csz])
                        nc.sync.dma_start(et[:], eT_d[:, c0:c0 + csz])
                        nc.sync.dma_start(
                            att[:], atT_d[:, ci * CHUNK_T * W:ci * CHUNK_T * W + ntl * W])
                        chunk_tiles[ci] = (xs, xd, et, att)
                    return chunk_tiles[ci], j

                g = 0
                for w in range(nwin):
                    win = pwin.tile([W, D + H], F32, tag="win")
                    tw = t_w[w]
                    t = 0
                    while t < tw:
                        grp = min(GRP, tw - t)
                        k4 = pk.tile([TE, grp * D], F32, tag="k4")
                        eq4 = peq.tile([TE, grp * 2 * D], F32, tag="eq4")
                        v4 = pv.tile([TE, grp * D], F32, tag="v4")
                        for j in range(grp):
                            (xs, xd, et, att), cj = get_chunk(g + j)
                            sl = slice(cj * TE, (cj + 1) * TE)
                            js = slice(j * D, (j + 1) * D)
                            nc.tensor.matmul(k4[:, js], xs[:, sl], wk[:])
                            nc.tensor.matmul(v4[:, js], xs[:, sl], wv[:])
                            nc.tensor.matmul(eq4[:, 2 * j * D:(2 * j + 1) * D],
                                             et[:, sl], we[:])
                            nc.tensor.matmul(eq4[:, (2 * j + 1) * D:(2 * j + 2) * D],
                                             xd[:, sl], wq[:])
                        # E and Q evicted in ONE ACT op (adjacent PSUM banks)
                        eqsb = espool.tile([TE, grp * 2 * D], BF16, tag="eqsb")
                        nc.scalar.activation(eqsb[:], eq4[:], COPY)
                        eqv = eqsb[:].rearrange("p (g s c) -> p g s c", g=grp, s=2)
                        t1 = espool.tile([TE, grp * D], BF16, tag="t1")
                        t1g = t1[:].rearrange("p (g c) -> p g c", g=grp)
                        nc.vector.tensor_tensor(t1g, k4[:].rearrange(
                            "p (g c) -> p g c", g=grp), eqv[:, :, 0, :], op=MULT)
                        t2 = espool.tile([TE, grp * D], BF16, tag="t2")
                        nc.vector.tensor_tensor(
                            t2[:].rearrange("p (g c) -> p g c", g=grp),
                            t1g, eqv[:, :, 1, :], op=MULT)
                        # per-head sum: pairwise tree on Pool (keeps DVE free;
                        # TensorReduce would be 1x on DVE anyway)
                        with nc.allow_low_precision("score logit rounding ok"):
                            t2v = t2[:].rearrange("p (g h d) -> p g h d", g=grp, h=H)
                            r1 = espool.tile([TE, grp * H * 8], BF16, tag="r1")
                            r1v = r1[:].rearrange("p (g h d) -> p g h d", g=grp, h=H)
                            nc.gpsimd.tensor_tensor(
                                r1v, t2v[:, :, :, 0:8], t2v[:, :, :, 8:16], op=ADD)
                            r2 = espool.tile([TE, grp * H * 4], BF16, tag="r2")
                            r2v = r2[:].rearrange("p (g h d) -> p g h d", g=grp, h=H)
                            nc.gpsimd.tensor_tensor(
                                r2v, r1v[:, :, :, 0:4], r1v[:, :, :, 4:8], op=ADD)
                            r3 = espool.tile([TE, grp * H * 2], BF16, tag="r3")
                            r3v = r3[:].rearrange("p (g h d) -> p g h d", g=grp, h=H)
                            nc.gpsimd.tensor_tensor(
                                r3v, r2v[:, :, :, 0:2], r2v[:, :, :, 2:4], op=ADD)
                            s4 = espool.tile([TE, grp * H], BF16, tag="s4")
                            s4v = s4[:].rearrange("p (g h) -> p g h", g=grp)
                            nc.gpsimd.tensor_tensor(
                                s4v.unsqueeze(3), r3v[:, :, :, 0:1], r3v[:, :, :, 1:2],
                                op=ADD)
                        nc.gpsimd.tensor_scalar(s4[:], s4[:], 20.0, -20.0,
                                                op0=MIN, op1=MAX)
                        ms4 = espool.tile([TE, grp * (D + H)], BF16, tag="ms4")
                        ms4g = ms4[:].rearrange("p (g x) -> p g x", g=grp)
                        nc.scalar.activation(
                            ms4g[:, :, D:D + H],
                            s4[:].rearrange("p (g h) -> p g h", g=grp),
                            EXP, bias=zb[:], scale=0.25)
                        # msg = V*score on DVE: V from PSUM, broadcast fused
                        scb = ms4g[:, :, D:D + H].unsqueeze(3) \
                            .broadcast_to([TE, grp, H, DH])
                        nc.vector.tensor_tensor(
                            ms4g[:, :, 0:D].rearrange("p g (h d) -> p g h d", h=H),
                            v4[:].rearrange("p (g h d) -> p g h d", g=grp, h=H),
                            scb, op=MULT)
                        for j in range(grp):
                            (xs, xd, et, att), cj = get_chunk(g + j)
                            nc.tensor.matmul(
                                win[:], att[:, cj * W:(cj + 1) * W],
                                ms4[:, j * (D + H):(j + 1) * (D + H)],
                                start=(t + j == 0), stop=(t + j == tw - 1))
                        gl = g + grp - 1
                        if gl % CHUNK_T == CHUNK_T - 1 or gl == ttot - 1:
                            chunk_tiles[gl // CHUNK_T] = None
                        g += grp
                        t += grp

                    # ---- window close: h = x + wV/(Z+eps); h^2 + bn1 stats
                    zi = espool.tile([W, H], F32, tag="zi")
                    nc.scalar.activation(zi[:], win[:, D:D + H], COPY, bias=EPS_Z)
                    nc.vector.reciprocal(zi[:], zi[:])
                    hw = h_all[:, w * D:(w + 1) * D]
                    nc.vector.tensor_tensor(
                        hw.rearrange("p (h d) -> p h d", h=H),
                        win[:, 0:D].rearrange("p (h d) -> p h d", h=H),
                        zi[:].unsqueeze(2).broadcast_to([W, H, DH]), op=MULT)
                    nc.vector.tensor_tensor(hw, hw,
                                            x_all[:, w * D:(w + 1) * D], op=ADD)
                    h3w = h3_all[:, w * D:(w + 1) * D]
                    nc.scalar.activation(h3w, hw, SQUARE)
                    st = (w == 0)
                    sp = (w == nwin - 1)
                    nc.tensor.matmul(ph1, masks[:, w:w + 1], hw,
                                     start=st, stop=sp)
                    nc.tensor.matmul(ph2, masks[:, w:w + 1], h3w,
                                     start=st, stop=sp)

            # ================= node phase =================
            with tc.tile_pool(name="nsmall", bufs=3) as nspool:

                def bn_coefs(gstat_ap, g_ap, b_ap, tag):
                    mu = nspool.tile([1, D], F32, tag=tag + "mu")
                    nc.vector.tensor_scalar_mul(mu[:], gstat_ap[:, 0:D], inv_n)
                    var = nspool.tile([1, D], F32, tag=tag + "var")
                    nc.vector.tensor_scalar_mul(var[:], gstat_ap[:, D:2 * D], inv_n)
                    mu2 = nspool.tile([1, D], F32, tag=tag + "mu2")
                    nc.vector.tensor_tensor(mu2[:], mu[:], mu[:], op=MULT)
                    nc.vector.tensor_tensor(var[:], var[:], mu2[:], op=SUB)
                    nc.vector.tensor_scalar_add(var[:], var[:], EPS_BN)
                    sd = nspool.tile([1, D], F32, tag=tag + "sd")
                    nc.scalar.activation(sd[:], var[:], SQRT, bias=zb[0:1, :])
                    nc.vector.reciprocal(sd[:], sd[:])
                    sg = nspool.tile([1, D], F32, tag=tag + "sg")
                    nc.vector.tensor_tensor(sg[:], sd[:], g_ap, op=MULT)
                    bb = nspool.tile([1, D], F32, tag=tag + "bb")
                    nc.vector.tensor_tensor(bb[:], mu[:], sg[:], op=MULT)
                    nc.vector.tensor_tensor(bb[:], b_ap, bb[:], op=SUB)
                    return sg, bb

                def stats_reduce(ph, ph2_, pbc, coef_g, coef_b, tag):
                    """AllReduce stats -> bn coefs, broadcast
                    [scale | bias] (bf16) to 128 partitions."""
                    stats = nspool.tile([1, 2 * D], F32, tag=tag + "stats")
                    nc.vector.tensor_copy(stats[:, 0:D], ph)
                    nc.vector.tensor_copy(stats[:, D:2 * D], ph2_)
                    cc_in = dpool.tile([1, 2 * D], F32, tag=tag + "ccin")
                    cc_out = dpool.tile([1, 2 * D], F32, tag=tag + "ccout")
                    nc.scalar.dma_start(cc_in[:], stats[:])
                    nc.gpsimd.collective_compute(
                        "AllReduce", ADD, replica_groups=[list(range(ncores))],
                        ins=[cc_in.opt()], outs=[cc_out.opt()])
                    gstat = nspool.tile([1, 2 * D], F32, tag=tag + "gstat")
                    nc.scalar.dma_start(gstat[:], cc_out[:])
                    sg, bb = bn_coefs(gstat, coef_g, coef_b, tag)
                    brc = nspool.tile([1, 2 * D], F32, tag=tag + "brc")
                    nc.vector.tensor_copy(brc[:, 0:D], sg[:])
                    nc.vector.tensor_copy(brc[:, D:2 * D], bb[:])
                    pb = pbc.tile([128, 2 * D], F32, tag=tag + "pb")
                    nc.tensor.matmul(pb[:], onesr[:], brc[:])
                    bc = nspool.tile([128, 2 * D], BF16, tag=tag + "bc")
                    nc.vector.tensor_copy(bc[:], pb[:])
                    return bc, sg, bb

                # h transposed (bf16) for the FFN, done OVERLAPPING the bn1
                # AllReduce: bn1's affine folds into W1 rows (scale by s1)
                # and the relu bias; the per-channel shifts (b1', b2) are
                # invariant under bn2 and are dropped from the residual path.
                hT_all = rpool.tile([D, nwin * W], BF16)

                with (
                    tc.tile_pool(name="pbc1", bufs=1, space="PSUM") as pbc1,
                    tc.tile_pool(name="ptr", bufs=2, space="PSUM") as ptr,
                ):
                    # stats were accumulated during the edge phase; the
                    # AllReduce launches ASAP and the transposes below
                    # overlap the collective latency
                    bc1, sg1, bb1 = stats_reduce(ph1, ph2, pbc1,
                                                 bnvec[:, 0:D], bnvec[:, D:2 * D],
                                                 "s1")
                    for w in range(nwin):
                        hw = h_all[:, w * D:(w + 1) * D]
                        pT = ptr.tile([D, W], BF16, tag="pT")
                        nc.tensor.transpose(pT[:], hw, identb[:])
                        nc.scalar.activation(hT_all[:, w * W:(w + 1) * W], pT[:],
                                             COPY)
                    # column versions of s1/b1' via K=1 matmuls
                    s1c_p = pbc1.tile([D, 1], F32, tag="s1cp")
                    nc.tensor.matmul(s1c_p[:], sg1[:], onesr[0:1, 0:1])
                    s1col = nspool.tile([D, 1], F32, tag="s1col")
                    nc.vector.tensor_copy(s1col[:], s1c_p[:])
                    b1c_p = pbc1.tile([D, 1], F32, tag="b1cp")
                    nc.tensor.matmul(b1c_p[:], bb1[:], onesr[0:1, 0:1])
                    b1col = nspool.tile([D, 1], BF16, tag="b1col")
                    nc.vector.tensor_copy(b1col[:], b1c_p[:])
                    # W1' = diag(s1) @ W1  (row scaling on ACT)
                    w1p = nspool.tile([D, 2 * D], BF16, tag="w1p")
                    nc.scalar.activation(w1p[:], w1[:], COPY, scale=s1col[:])
                    # relu bias: b1' @ W1 + b1, as [D, 2] columns
                    bias2 = nspool.tile([D, 2], F32, tag="bias2")
                    for jj in range(2):
                        bv_p = pbc1.tile([D, 1], F32, tag="bvp")
                        nc.tensor.matmul(bv_p[:], w1[:, jj * D:(jj + 1) * D],
                                         b1col[:])
                        nc.vector.tensor_tensor(bias2[:, jj:jj + 1], bv_p[:],
                                                b1t[:, jj:jj + 1], op=ADD)

                # ---- Phase 4: FFN + residual (h*s1 + h2) + bn2 stats ----
                with tc.tile_pool(name="pstat2", bufs=1, space="PSUM") as pstat2:
                  with tc.tile_pool(name="pffn", bufs=2, space="PSUM") as pffn:
                    p2h = pstat2.tile([1, D], F32, tag="s2ph")
                    p2h2 = pstat2.tile([1, D], F32, tag="s2ph2")
                    for w in range(nwin):
                        hw = h_all[:, w * D:(w + 1) * D]
                        h3w = h3_all[:, w * D:(w + 1) * D]
                        sq2w = x_all[:, w * D:(w + 1) * D]
                        hnT = hT_all[:, w * W:(w + 1) * W]
                        f1a = pffn.tile([D, W], F32, tag="f1a")
                        f1b = pffn.tile([D, W], F32, tag="f1b")
                        nc.tensor.matmul(f1a[:], w1p[:, 0:D], hnT)
                        nc.tensor.matmul(f1b[:], w1p[:, D:2 * D], hnT)
                        ra = nspool.tile([D, W], BF16, tag="ra")
                        rb = nspool.tile([D, W], BF16, tag="rb")
                        nc.scalar.activation(ra[:], f1a[:], RELU, bias=bias2[:, 0:1])
                        nc.scalar.activation(rb[:], f1b[:], RELU, bias=bias2[:, 1:2])
                        # h2 node-major directly: lhsT = r (stationary),
                        # rhs = W2 rows -> out [n, c]; no transpose needed
                        p2 = pffn.tile([W, D], F32, tag="p2")
                        nc.tensor.matmul(p2[:], ra[:], w2[:, 0:D],
                                         start=True, stop=False)
                        nc.tensor.matmul(p2[:], rb[:], w2[:, D:2 * D],
                                         start=False, stop=True)
                        # h3' = h*s1 + h2   (b1', b2 shifts dropped: bn2-invariant)
                        nc.vector.tensor_tensor(h3w, hw, bc1[:, 0:D], op=MULT)
                        nc.vector.tensor_tensor(h3w, h3w, p2[:], op=ADD)
                        nc.gpsimd.tensor_tensor(sq2w, h3w, h3w, op=MULT)
                        st = (w == 0)
                        sp = (w == nwin - 1)
                        nc.tensor.matmul(p2h[:], masks[:, w:w + 1], h3w,
                                         start=st, stop=sp)
                        nc.tensor.matmul(p2h2[:], masks[:, w:w + 1], sq2w,
                                         start=st, stop=sp)
                  with tc.tile_pool(name="pbc2", bufs=1, space="PSUM") as pbc2:
                    bc2, _, _ = stats_reduce(p2h[:], p2h2[:], pbc2,
                                             bnvec[:, 2 * D:3 * D],
                                             bnvec[:, 3 * D:4 * D], "s2")

                # ---- Phase 6: bn2 apply (batched, into h_all) + one DMA out ----
                PCH = 13
                for w0 in range(0, nwin, PCH):
                    w1_ = min(w0 + PCH, nwin)
                    hs = h_all[:, w0 * D:w1_ * D].rearrange(
                        "p (w c) -> p w c", w=w1_ - w0)
                    h3s = h3_all[:, w0 * D:w1_ * D].rearrange(
                        "p (w c) -> p w c", w=w1_ - w0)
                    nb = w1_ - w0
                    nc.vector.tensor_tensor(
                        hs, h3s, bc2[:, 0:D].unsqueeze(1).broadcast_to([128, nb, D]),
                        op=MULT)
                    nc.vector.tensor_tensor(
                        hs, hs, bc2[:, D:2 * D].unsqueeze(1).broadcast_to([128, nb, D]),
                        op=ADD)
                    nc.scalar.dma_start(
                        out_d[w0 * W:w1_ * W, :].rearrange("(w p) c -> p w c", p=W),
                        hs)

    nc.compile()
    return nc


def pack_windows(dst_local, npc, nwin):
    """Greedy LPT: assign nodes to windows (<=W nodes each) balancing
    per-window edge counts. Returns (node_window, node_slot)."""
    deg = np.bincount(dst_local, minlength=npc)
    order = np.argsort(-deg, kind="stable")
    loads = np.zeros(nwin, dtype=np.int64)
    counts = np.zeros(nwin, dtype=np.int64)
    node_window = np.empty(npc, dtype=np.int64)
    node_slot = np.empty(npc, dtype=np.int64)
    # per-window max nodes: spread the remainder so no window is starved
    cap = np.full(nwin, W, dtype=np.int64)
    open_set = list(range(nwin))
    import heapq
    heap = [(0, w) for w in range(nwin)]
    heapq.heapify(heap)
    for n in order:
        while True:
            load, wsel = heapq.heappop(heap)
            if counts[wsel] < cap[wsel]:
                break
        node_window[n] = wsel
        node_slot[n] = counts[wsel]
        counts[wsel] += 1
        loads[wsel] += deg[n]
        if counts[wsel] < cap[wsel]:
            heapq.heappush(heap, (loads[wsel], wsel))
    return node_window, node_slot, counts


def host_prepare(x, edge_attr, edge_index, Wq, Wk, We, Wv, bn1_g, bn1_b,
                 W1, b1, W2, b2, bn2_g, bn2_b, ncores):
    """Shard + permute inputs on the host; returns (in_maps, nwin, t_w, N, perm)."""
    N = x.shape[0]
    E = edge_index.shape[1]
    assert N % ncores == 0, (N, ncores)
    npc = N // ncores
    nwin = (npc + W - 1) // W
    npcp = nwin * W

    src = np.asarray(edge_index[0], dtype=np.int64)
    dst = np.asarray(edge_index[1], dtype=np.int64)
    core = dst // npc
    rem = dst - core * npc

    # per-core greedy node->window packing to balance edges per window
    node_window = np.empty(N, dtype=np.int64)
    node_slot = np.empty(N, dtype=np.int64)
    wcounts = np.zeros((ncores, nwin), dtype=np.int64)
    for c in range(ncores):
        m = core == c
        nw, ns, cnt = pack_windows(rem[m], npc, nwin)
        node_window[c * npc:(c + 1) * npc] = nw
        node_slot[c * npc:(c + 1) * npc] = ns
        wcounts[c] = cnt

    wloc = node_window[dst]
    dloc = node_slot[dst]

    gw = core * nwin + wloc  # global window id in [0, ncores*nwin)
    order = np.argsort(gw, kind="stable")
    gw_s = gw[order]
    counts = np.bincount(gw_s, minlength=ncores * nwin).reshape(ncores, nwin)
    t_w = np.maximum(1, (counts.max(axis=0) + TE - 1) // TE).astype(np.int64)
    ttot = int(t_w.sum())
    etot = ttot * TE

    tile_off = np.concatenate([[0], np.cumsum(t_w)])  # per window, in tiles

    # slot index for every edge: position within its (core, window) group
    starts = np.concatenate([[0], np.cumsum(counts.reshape(-1))])
    within = np.arange(E, dtype=np.int64) - starts[gw_s]
    slot = (core[order] * etot + tile_off[wloc[order]] * TE + within)

    x32 = np.asarray(x, dtype=np.float32)
    ea32 = np.asarray(edge_attr, dtype=np.float32)

    # padded per-core edge-slot arrays
    xsrc = np.zeros((ncores, etot, D), dtype=np.float32)
    xdst = np.zeros((ncores, etot, D), dtype=np.float32)
    eat = np.zeros((ncores, etot, D), dtype=np.float32)
    dlocs = np.full((ncores, etot), -1.0, dtype=np.float32)
    sc_idx = slot // etot
    sl_idx = slot % etot
    xsrc[sc_idx, sl_idx] = x32[src[order]]
    xdst[sc_idx, sl_idx] = x32[dst[order]]
    eat[sc_idx, sl_idx] = ea32[order]
    dlocs[sc_idx, sl_idx] = dloc[order].astype(np.float32)

    identb = np.eye(128, dtype=np.float32).astype(BF)
    bnvec = np.concatenate([bn1_g, bn1_b, bn2_g, bn2_b, b2]).reshape(1, 5 * D)
    bnvec = np.ascontiguousarray(bnvec, dtype=np.float32)
    b1t = np.ascontiguousarray(np.asarray(b1, np.float32).reshape(2, D).T)

    in_maps = []
    for c in range(ncores):
        # node-major x in packed (window, slot) order
        xl = np.zeros((npcp, D), dtype=np.float32)
        nid = np.arange(npc)
        flat_slot = node_window[c * npc:(c + 1) * npc] * W + \
            node_slot[c * npc:(c + 1) * npc]
        xl[flat_slot] = x32[c * npc + nid]
        masks = np.zeros((W, nwin), dtype=np.float32)
        for wv_ in range(nwin):
            masks[:wcounts[c, wv_], wv_] = 1.0
        in_maps.append({
            "xsrcT": np.ascontiguousarray(xsrc[c].T).astype(BF),
            "xdstT": np.ascontiguousarray(xdst[c].T).astype(BF),
            "eT": np.ascontiguousarray(eat[c].T).astype(BF),
            "atT": np.ascontiguousarray(
                (dlocs[c].reshape(ttot, TE)[:, :, None]
                 == np.arange(W, dtype=np.float32)[None, None, :])
                .transpose(1, 0, 2).reshape(TE, ttot * W)).astype(ml_dtypes.float8_e4m3),
            "x_nm": xl.astype(BF),
            "wk": np.asarray(Wk, np.float32).astype(BF),
            "wv": np.asarray(Wv, np.float32).astype(BF),
            "wq": np.asarray(Wq, np.float32).astype(BF),
            "we": np.asarray(We, np.float32).astype(BF),
            "w1": np.asarray(W1, np.float32).astype(BF),
            "w2": np.concatenate(
                [np.asarray(W2, np.float32)[0:D], np.asarray(W2, np.float32)[D:2 * D]],
                axis=1).astype(BF),
            "b1t": b1t,
            "masks": masks.astype(BF),
            "bnvec": bnvec,
            "identb": identb,
        })
    return in_maps, nwin, [int(t) for t in t_w], N, (node_window, node_slot)


_CACHE = {}


def _get_program(ncores, nwin, t_w, n_total):
    key = (ncores, nwin, tuple(t_w), n_total)
    if key not in _CACHE:
        _CACHE[key] = build_program(ncores, nwin, t_w, n_total)
    return _CACHE[key]


def kernel(x, edge_attr, edge_index, Wq, Wk, We, Wv, bn1_g, bn1_b,
           W1, b1, W2, b2, bn2_g, bn2_b, _ncores=8, _return_extra=False):
    from concourse.bass_utils import run_bass_kernel_spmd

    in_maps, nwin, t_w, N, (node_window, node_slot) = host_prepare(
        x, edge_attr, edge_index, Wq, Wk, We, Wv, bn1_g, bn1_b,
        W1, b1, W2, b2, bn2_g, bn2_b, _ncores)
    nc = _get_program(_ncores, nwin, t_w, N)
    res = run_bass_kernel_spmd(nc, in_maps, list(range(_ncores)))
    npc = N // _ncores
    flat_slot = node_window * W + node_slot
    out = np.concatenate(
        [np.asarray(r["out"], dtype=np.float32)[flat_slot[c * npc:(c + 1) * npc]]
         for c, r in enumerate(res.results)], axis=0)
    out = np.ascontiguousarray(out, dtype=np.float32)
    if _return_extra:
        return out, res
    return out
